# revision 29
# baseline (speedup 1.0000x reference)
"""Trainium2 Bass kernel for CantorMultiheadFusion.

Reference math:
    h      = x @ W_in^T                        # [B,S,D]
    d[s,k] = distances[s, routes[s,k]]
    w      = softmax(-d, axis=-1)              # [S,K]
    fused  = sum_k w[s,k] * h[:, routes[s,k]]  # [B,S,D]  (head reshape is a no-op)
    out    = fused @ W_out^T + b_out + x

Because the fusion weights are shared across the feature dim, the gather
commutes with both projections:
    out = (A @ x) @ (W_out @ W_in)^T + b_out + x
where A[s,j] = C[s,j] * exp(-distances[s,j]) / denom(s),
      C[s,j] = #{k : routes[s,k] == j}   (integer multiplicity),
      denom(s) = sum_j C[s,j] * exp(-distances[s,j]).
Duplicated route entries share the same distance, so the count matrix C is
exact. On device this is computed as exp(ln(C) - D) with ln(C) built
host-side from the int32 routes tensor alone (index marshalling; C=0 maps
to -448 so exp underflows to +0). All float math on the actual inputs
(exp, normalization, matmuls, residual) runs on device.

Sharding: sequence-parallel over S across 8 cores (256 rows each). x is
replicated since the A@x contraction needs all S rows.

The matmul datapath runs in fp8e4m3 with DoubleRow perf mode (two
contraction tiles per instruction, 0.5 PE cycles/row):
  stepA: t^T[e,s] += x[b,jpair](lhsT) @ numer[jpair](rhs)    (j contract)
  Wc   : Wc[e,i]  = sum_a (16*W_in)[a,e] * W_outT[a,i]       (a contract)
  stepB: out[s,i] += tT[epair](lhsT) @ Wc[epair](rhs)        (e contract)
fp8 range handling: W_in is pre-scaled by 16 on host (exact power of 2) so
Wc sits in the fp8e4m3 sweet spot; the factor is folded into the softmax
denominator by using 16-valued ones in the denominator matmul (exact in
fp8), so the epilogue's rdT multiply removes it for free. t is
pre-normalization (sigma ~5-10) and ships through fp8 unscaled.

The numerator stream ships as ONE packed fp8 tensor per core
[S, lnct(256) | distT(256)] so every DMA row is 512B (full wire rate) and
each j-tile pair needs a single DMA/sub/exp. numer is fp8: the softmax
denominator is summed from the SAME fp8 values, so correlated quantization
error cancels in the normalized weights.

Schedule: the kernel is wire-bound (DMA bytes/360GB/s ~ 11.6us of input),
so the batches are software-pipelined: the full b0 pass (stream, stepA,
tT copies, stepB, epilogue, output) runs while b1's x stream is still on
the wire, hiding half the epilogue latency. All input DMAs ride one queue
(SP) in explicit wire order: ld0|xb0a|ld1|ld2|ld3|xb0b|W|xres|xb1a|xb1b.
Output is written bf16 and upcast to f32 on the host (pure dtype
marshalling) to halve output wire time.
"""

import os
import sys

import numpy as np

for _p in ("/opt/trn_rl_repo",):
    if os.path.isdir(_p) and _p not in sys.path:
        sys.path.insert(0, _p)

# Some container snapshots lack antenv.axon_hooks (the axon NTFF profile
# hook); stub it so run_bass_kernel_spmd(trace=True) degrades gracefully.
def _ensure_axon_hooks_stub():
    import types
    try:
        import antenv.axon_hooks  # noqa: F401
    except ModuleNotFoundError:
        try:
            import antenv
        except ModuleNotFoundError:
            return
        _stub = types.ModuleType("antenv.axon_hooks")
        _stub.get_axon_ntff_profile_hook = lambda: None
        sys.modules["antenv.axon_hooks"] = _stub
        antenv.axon_hooks = _stub


_ensure_axon_hooks_stub()

B, S, D, K = 2, 2048, 512, 64
N_CORES = 8
SLOC = S // N_CORES          # 256 sequence rows per core
NJ = S // 128                # 16 contraction tiles
NP = NJ // 2                 # 8 DoubleRow contraction pairs
NE = D // 128                # 4 feature chunks
NSC = SLOC // 128            # 2 seq chunks per core

WSCALE = 16.0                # host scale on W_in (exact power of 2)
ONES_VAL = WSCALE           # folded into the softmax denominator

LD_GROUPS = [(0, 2), (2, 2), (4, 2), (6, 2)]   # numerator stream (pairs)
# x stream groups per batch; b1's tail shrinks so the last stepA waits on
# as little wire as possible
XG = {0: [(0, 4), (4, 4)], 1: [(0, 4), (4, 2), (6, 2)]}

_CACHE = {}
LAST_RESULTS = None


def _build_nc(with_bias=True):
    import concourse.bacc as bacc
    import concourse.mybir as mybir
    import concourse.tile as tile

    F32 = mybir.dt.float32
    BF16 = mybir.dt.bfloat16
    F8 = mybir.dt.float8e4
    MUL = mybir.AluOpType.mult
    ADD = mybir.AluOpType.add
    DR = mybir.MatmulPerfMode.DoubleRow
    EXP = mybir.ActivationFunctionType.Exp

    nc = bacc.Bacc("TRN2", target_bir_lowering=False, debug=False, num_devices=1)

    x_d = nc.dram_tensor("x", [B, S, D], F8, kind="ExternalInput").ap()
    ld_d = nc.dram_tensor("ldpack", [S, 2 * SLOC], F8, kind="ExternalInput").ap()
    xres_d = nc.dram_tensor("xres", [B, SLOC, D], BF16, kind="ExternalInput").ap()
    wpack_d = nc.dram_tensor("wpack", [2, D, D], F8, kind="ExternalInput").ap()
    bout_d = nc.dram_tensor("b_out", [1, D], F32, kind="ExternalInput").ap()
    out_d = nc.dram_tensor("out", [B, SLOC, D], BF16, kind="ExternalOutput").ap()

    with tile.TileContext(nc) as tc:
        with (
            tc.tile_pool(name="big", bufs=1) as big,
            tc.tile_pool(name="ldstream", bufs=4) as ldstream,
            tc.tile_pool(name="sstream", bufs=2) as sstream,
            tc.tile_pool(name="pa", bufs=4, space="PSUM") as pa,
            tc.tile_pool(name="pden", bufs=1, space="PSUM") as pdenp,
            tc.tile_pool(name="ptr", bufs=2, space="PSUM") as ptr,
        ):
            # ---- persistent SBUF ----
            # one tile per x DMA group so a group's consumers never gate on
            # later groups' DMAs (dependency tracking is per-tile)
            xb = {(b, g): big.tile([128, 2 * pn, D], F8, name=f"xb{b}_{g}")
                  for b in range(B) for g, (p0, pn) in enumerate(XG[b])}
            numer = big.tile([128, NJ, SLOC], F8)       # [128j, j, 256s]
            w_sb = big.tile([128, 8, D], F8)            # [128a, (w,t), *]
            wcT = big.tile([128, NE, D], F8)            # [128e, ec, 512i]
            ones2 = big.tile([128, 2], F8)
            xres_sb = big.tile([128, B * NSC - 1, D], BF16)
            xres_last = big.tile([128, 1, D], BF16)
            tT = big.tile([128, B * NE, SLOC], F8)      # [128e, (b,ec), 256s]
            outbuf = big.tile([128, B * NSC, D], BF16)
            rdT = big.tile([128, NSC], F32)
            if with_bias:
                bias_sb = big.tile([1, D], F32)
                bias_bc = big.tile([128, D], F32)
                ones_r = big.tile([1, 128], F32)
                resb = big.tile([128, B * NSC, D], F32)

            nc.vector.memset(ones2[:], ONES_VAL)

            # stepA PSUM: bank (b,p) holds ec=2p (cols :SLOC) and ec=2p+1
            # (cols SLOC:). Exactly one start=True per bank.
            pdT = pdenp.tile([128, NSC], F32)
            pts = {(b, p): pa.tile([128, 2 * SLOC], F32,
                                   name=f"pts{b}_{p}", tag="acc")
                   for b in range(B) for p in range(NE // 2)}

            def pta(b, ec):
                return pts[(b, ec // 2)][:, (ec % 2) * SLOC:(ec % 2 + 1) * SLOC]

            if with_bias:
                nc.scalar.dma_start(out=bias_sb[:1, :], in_=bout_d[:, :])
                nc.vector.memset(ones_r[:], 1.0)
                pb = ptr.tile([128, D], F32, name="pb", tag="tr")
                nc.tensor.matmul(pb[:], lhsT=ones_r[:1, :], rhs=bias_sb[:1, :],
                                 start=True, stop=True)
                nc.vector.tensor_copy(bias_bc[:], pb[:])

            def npair(pp):
                return numer[:, 2 * pp:2 * pp + 2, :]

            def denom_mm(pp):
                n3 = npair(pp)
                for sc in range(NSC):
                    nc.tensor.matmul(
                        pdT[:, sc:sc + 1],
                        lhsT=n3[:, :, sc * 128:(sc + 1) * 128],
                        rhs=ones2[:].rearrange("p (j o) -> p j o", j=2),
                        start=(pp == 0 and sc == 0),
                        stop=(pp == NP - 1 and sc == NSC - 1),
                        perf_mode=DR, skip_group_check=True)

            def stepa(pp, b):
                for g, (p0, pn) in enumerate(XG[b]):
                    if p0 <= pp < p0 + pn:
                        x3 = xb[(b, g)][:, 2 * (pp - p0):2 * (pp - p0) + 2, :]
                        break
                n3 = npair(pp)
                for ec in range(NE):
                    nc.tensor.matmul(
                        pta(b, ec),
                        lhsT=x3[:, :, ec * 128:(ec + 1) * 128],
                        rhs=n3,
                        start=(pp == 0 and ec % 2 == 0),
                        stop=(pp == NP - 1 and ec % 2 == 1),
                        perf_mode=DR, skip_group_check=True)

            def tt_copies(b):
                # PSUM->SBUF fp8, one bank on DVE and one on ACT in parallel
                nc.vector.tensor_copy(tT[:, b * NE:b * NE + 2, :],
                                      pts[(b, 0)][:])
                nc.scalar.copy(tT[:, b * NE + 2:b * NE + 4, :],
                               pts[(b, 1)][:])

            def x_dma(b, g):
                xp0, xpn = XG[b][g]
                nc.sync.dma_start(
                    out=xb[(b, g)][:],
                    in_=x_d[b, 2 * xp0 * 128:2 * (xp0 + xpn) * 128, :]
                        .rearrange("(j p) e -> p j e", p=128))

            def stepb_mm(b, sc):
                po = pa.tile([128, D], F32, name=f"po{b}_{sc}", tag="acc")
                t3 = tT[:, b * NE:(b + 1) * NE, :]
                for ep in range(2):
                    nc.tensor.matmul(
                        po[:],
                        lhsT=t3[:, 2 * ep:2 * ep + 2, sc * 128:(sc + 1) * 128],
                        rhs=wcT[:, 2 * ep:2 * ep + 2, :],
                        start=(ep == 0), stop=(ep == 1),
                        perf_mode=DR)
                return po

            def epilogue(b, sc, po, out_eng, via_act=False):
                if with_bias:
                    res, ri = resb, b * NSC + sc
                elif b * NSC + sc == B * NSC - 1:
                    res, ri = xres_last, 0
                else:
                    res, ri = xres_sb, b * NSC + sc
                if via_act:
                    # ACT scales, DVE adds: overlaps the sibling chunk's stt
                    tmp = big.tile([128, D], BF16, name=f"tmp{b}_{sc}")
                    nc.scalar.activation(tmp[:], po[:],
                                         mybir.ActivationFunctionType.Copy,
                                         scale=rdT[:, sc:sc + 1])
                    nc.vector.tensor_add(outbuf[:, b * NSC + sc, :],
                                         tmp[:], res[:, ri, :])
                else:
                    nc.vector.scalar_tensor_tensor(
                        out=outbuf[:, b * NSC + sc, :],
                        in0=po[:],
                        scalar=rdT[:, sc:sc + 1],
                        in1=res[:, ri, :],
                        op0=MUL, op1=ADD)
                out_eng.dma_start(
                    out=out_d[b, sc * 128:(sc + 1) * 128, :],
                    in_=outbuf[:, b * NSC + sc, :])

            # ---- input stream DMAs upfront on SP, in explicit wire order:
            # ld0 | xb0a | ld1 ld2 ld3 | xb0b | xb1... W/xres ride the ACT
            # queue (emitted between exps) so their consumers don't couple to
            # the SP stream counter and their wire slots land mid-stream ----
            ld_ts = []
            for g, (p0, pn) in enumerate(LD_GROUPS):
                jn, j0 = 2 * pn, 2 * p0
                ld_t = ldstream.tile([128, jn, 2 * SLOC], F8,
                                     name=f"ld{g}", tag="ld")
                ld_ts.append(ld_t)
                nc.sync.dma_start(
                    out=ld_t[:],
                    in_=ld_d[j0 * 128:(j0 + jn) * 128, :]
                        .rearrange("(j p) c -> p j c", p=128))
            x_dma(0, 0)
            x_dma(0, 1)
            nc.sync.dma_start(
                out=w_sb[:],
                in_=wpack_d.rearrange("w (t p) e -> p (w t) e", p=128))
            # residual ships in two pieces: the b1/sc1 block goes LAST on the
            # wire — its only consumer is the final 0.3us add, the shortest
            # possible tail chain, so the wire end gates almost nothing
            xres_flat = xres_d.rearrange("b (sc p) e -> p (b sc) e", p=128)
            nc.sync.dma_start(out=xres_sb[:], in_=xres_flat[:, 0:3, :])
            for g in range(len(XG[1])):
                x_dma(1, g)
            nc.sync.dma_start(out=xres_last[:], in_=xres_flat[:, 3:4, :])

            # ---- numerator + b0 stepA pass ----
            for g, (p0, pn) in enumerate(LD_GROUPS):
                ld_t = ld_ts[g]
                jn = 2 * pn
                # one batched sub+exp per group: fewer fixed-overhead slots
                # on the serial DVE/ACT chains
                sb_t = sstream.tile([128, jn, SLOC], BF16,
                                    name=f"sb{g}", tag="sb")
                nc.vector.tensor_sub(sb_t[:], ld_t[:, :, :SLOC],
                                     ld_t[:, :, SLOC:])
                nc.scalar.activation(numer[:, 2 * p0:2 * p0 + jn, :],
                                     sb_t[:], EXP)
                for q in range(pn):
                    pp = p0 + q
                    denom_mm(pp)
                    stepa(pp, 0)

            tt_copies(0)
            nc.vector.reciprocal(rdT[:], pdT[:])

            # Wc[e,i] = sum_a (16*W_in)[a,e] * W_outT[a,i], fp8 DoubleRow
            for ec in range(NE):
                pw = ptr.tile([128, D], F32, name=f"pw{ec}", tag="tr")
                for ap_ in range(2):
                    nc.tensor.matmul(
                        pw[:],
                        lhsT=w_sb[:, 2 * ap_:2 * ap_ + 2,
                                  ec * 128:(ec + 1) * 128],
                        rhs=w_sb[:, 4 + 2 * ap_:4 + 2 * ap_ + 2, :],
                        start=(ap_ == 0), stop=(ap_ == 1),
                        perf_mode=DR)
                if ec < 2:
                    nc.vector.tensor_copy(wcT[:, ec, :], pw[:])
                else:
                    nc.scalar.copy(wcT[:, ec, :], pw[:])

            if with_bias:
                for b in range(B):
                    for sc in range(NSC):
                        i = b * NSC + sc
                        src_t = xres_last if i == B * NSC - 1 else xres_sb
                        nc.vector.tensor_add(resb[:, i, :],
                                             src_t[:, 0 if i == B * NSC - 1
                                                   else i, :],
                                             bias_bc[:])

            # ---- b0 stepB/epilogue overlapped with b1 stepA ----
            for pp in range(4):
                stepa(pp, 1)
            po00 = stepb_mm(0, 0)
            po01 = stepb_mm(0, 1)
            epilogue(0, 0, po00, nc.sync)
            epilogue(0, 1, po01, nc.sync, via_act=True)
            for pp in range(4, NP):
                stepa(pp, 1)
            tt_copies(1)
            po10 = stepb_mm(1, 0)
            po11 = stepb_mm(1, 1)
            epilogue(1, 0, po10, nc.scalar)
            epilogue(1, 1, po11, nc.sync, via_act=True)

    nc.compile()
    return nc


def _get_nc(with_bias=True):
    key = ("nc", with_bias)
    if key not in _CACHE:
        _CACHE[key] = _build_nc(with_bias)
    return _CACHE[key]


def prep_in_maps(x, routes, distances, W_in, W_out, b_out):
    """Host-side sharding/marshalling: per-core input dicts."""
    import ml_dtypes
    import concourse.mybir as mybir

    bf16 = ml_dtypes.bfloat16
    f8 = mybir.dt.np(mybir.dt.float8e4)
    x = np.ascontiguousarray(np.asarray(x, dtype=np.float32))
    routes = np.asarray(routes, dtype=np.int32)
    distances = np.ascontiguousarray(np.asarray(distances, dtype=np.float32))
    b_out = np.ascontiguousarray(np.asarray(b_out, dtype=np.float32)).reshape(1, D)

    wpack = np.empty((2, D, D), dtype=np.float32)
    wpack[0] = np.asarray(W_in, dtype=np.float32) * WSCALE
    wpack[1] = np.asarray(W_out, dtype=np.float32).T
    wpack = wpack.astype(f8)

    x_8 = x.astype(f8)
    xres_b = x.astype(bf16)

    # Count matrix C^T[j, s] = multiplicity of j in routes[s, :], shipped as
    # ln(C) so the device computes C*exp(-d) = exp(lnC - d); C=0 -> -448
    # underflows exp to +0. Depends only on the int32 index tensor.
    flat = routes.astype(np.int64).ravel() * S + np.repeat(np.arange(S, dtype=np.int64), K)
    countsT = np.bincount(flat, minlength=S * S).reshape(S, S)
    with np.errstate(divide="ignore"):
        lnctT = np.log(countsT.astype(np.float32))
    lnctT[countsT == 0] = -448.0
    distT = distances.T

    in_maps = []
    for c in range(N_CORES):
        sl = slice(c * SLOC, (c + 1) * SLOC)
        ldpack = np.empty((S, 2 * SLOC), dtype=np.float32)
        ldpack[:, :SLOC] = lnctT[:, sl]
        ldpack[:, SLOC:] = distT[:, sl]
        in_maps.append({
            "x": x_8,
            "ldpack": ldpack.astype(f8),
            "xres": np.ascontiguousarray(xres_b[:, sl, :]),
            "wpack": wpack,
            "b_out": b_out,
        })
    return in_maps


def kernel(x, routes, distances, W_in, W_out, b_out):
    global LAST_RESULTS
    from concourse import bass_utils

    in_maps = prep_in_maps(x, routes, distances, W_in, W_out, b_out)
    with_bias = bool(np.any(np.asarray(b_out)))
    nc = _get_nc(with_bias)
    _CACHE["last_nc"] = nc
    res = bass_utils.run_bass_kernel_spmd(nc, in_maps, core_ids=list(range(N_CORES)))
    LAST_RESULTS = res
    out = np.concatenate(
        [res.results[c]["out"].astype(np.float32) for c in range(N_CORES)],
        axis=1)
    return out


if __name__ == "__main__":
    rng = np.random.default_rng(0)
    inputs = {
        "x": rng.standard_normal((B, S, D), dtype=np.float32),
        "routes": rng.integers(0, S, (S, K)).astype(np.int32),
        "distances": rng.random((S, S), dtype=np.float32),
        "W_in": (rng.standard_normal((D, D), dtype=np.float32) / np.sqrt(D)).astype(np.float32),
        "W_out": (rng.standard_normal((D, D), dtype=np.float32) / np.sqrt(D)).astype(np.float32),
        "b_out": np.zeros(D, dtype=np.float32),
    }
    out = kernel(**inputs)
    print("out", out.shape, out.dtype)
